# revision 8
# baseline (speedup 1.0000x reference)
"""BitLinear inference kernel for Trainium2, sharded over 8 NeuronCores.

Computes, per the reference:
    w_q = sign(w - mean(w));  w_scale = mean(|w|)
    b_q = sign(b - mean(b));  b_scale = mean(|b|)
    xn  = x / max(||x||_2, 1e-12) * D**-0.5            (per token)
    sc  = 127 / max(max|xn|, 1e-5)                     (per token)
    x_q = clip(round(xn * sc), -128, 127)
    y   = (x_q @ w_q.T + b_q) / (w_scale * sc * b_scale)

Sharding: x/y split into 8 contiguous row blocks of 4096 tokens (data
parallel over B*S); w, b replicated.  All per-token math is on-core.

Implementation notes:
  - round(xn*sc) == round(x * 127/amax|x|) mathematically (the l2 norm
    cancels); fp-path differences only flip values sitting exactly on a
    rounding boundary (isolated x_q entries move by +-1; benign).
  - round-half-to-even done exactly with the +-1.5*2^23 magic constant.
  - x_q in bf16 (integers |v|<=127 exact) and w_q in bf16 ({-1,0,1})
    make the PE matmul bit-exact vs the f32 reference einsum.
  - x_q transpose: one DMA xbar transpose per token tile with a 3D
    destination [128, 8, 128]: out[p,c,t] = in[t, c*128+p] (verified on
    HW), which is exactly the per-chunk lhsT layout the matmul needs.
  - w transposed on PE in f32 BEFORE quantization (no stats dependency),
    then Sign(wT - mean) on ACT doubles as the PSUM->SBUF copy.
  - bias b_q is added via a K=1 rank-1 matmul accumulated into PSUM.
  - dequant scale needs 1/||x||: DVE reciprocal + ACT sqrt seed, then
    two Newton rsqrt refinements (ACT sqrt alone is too inaccurate).
"""

import os
import sys

import numpy as np

for _p in ("/opt/trn_rl_repo", "/root/.axon_site/_ro/trn_rl_repo"):
    if os.path.isdir(_p) and _p not in sys.path:
        sys.path.insert(0, _p)

import concourse.bacc as bacc
import concourse.bass_isa as bass_isa
import concourse.tile as tile
from concourse import mybir
from concourse.bass_utils import run_bass_kernel_spmd
from concourse.masks import make_identity

F32 = mybir.dt.float32
BF16 = mybir.dt.bfloat16
ALU = mybir.AluOpType
ACTF = mybir.ActivationFunctionType

N_CORES = 8
B, S, D, O = 4, 8192, 1024, 1024
TOKENS = B * S
TOK_PER_CORE = TOKENS // N_CORES          # 4096
P = 128                                   # partitions / token tile
NTILES = TOK_PER_CORE // P                # 32
GROUP = 4                                 # token tiles per stats group
NGROUPS = NTILES // GROUP
DCH = D // P                              # 8 contraction chunks
OCH = O // P                              # 8 weight row tiles

MAGIC = 1.5 * 2.0**23                     # round-to-nearest-even constant
DIM_SCALE = float(D) ** -0.5              # 2**-5, exact power of two
EPS_NORM_SQ = 1e-24                       # (1e-12)**2, matches l2 clamp
EPS_SCALE = 1e-5

# "xbar" (DMA crossbar) or "pe" path for the per-tile x_q transpose
TRANSPOSE_MODE = os.environ.get("BITLIN_TRANSPOSE", "xbar")


def build_module(repeat: int = 1):
    nc = bacc.Bacc("TRN2", target_bir_lowering=False, debug=False)

    x_d = nc.dram_tensor("x", [TOK_PER_CORE, D], F32, kind="ExternalInput")
    w_d = nc.dram_tensor("w", [O, D], F32, kind="ExternalInput")
    b_d = nc.dram_tensor("b", [O], F32, kind="ExternalInput")
    y_d = nc.dram_tensor("y", [TOK_PER_CORE, O], F32, kind="ExternalOutput")

    x_r = x_d.ap().rearrange("(a p) d -> p a d", p=P)   # [128, 32, 1024]
    y_r = y_d.ap().rearrange("(a p) d -> p a d", p=P)
    w_r = w_d.ap().rearrange("(r p) d -> p r d", p=P)   # [128, 8, 1024]
    b_r = b_d.ap().rearrange("(o d) -> o d", o=1)       # [1, 1024]

    with tile.TileContext(nc) as tc:
        import contextlib

        with contextlib.ExitStack() as ctx:
            consts = ctx.enter_context(tc.tile_pool(name="consts", bufs=1))
            wpool = ctx.enter_context(tc.tile_pool(name="wpool", bufs=1))
            wtpool = ctx.enter_context(tc.tile_pool(name="wtpool", bufs=1))
            xpool = ctx.enter_context(tc.tile_pool(name="xpool", bufs=3))
            scr = ctx.enter_context(tc.tile_pool(name="scr", bufs=2))
            tpool = ctx.enter_context(tc.tile_pool(name="tpool", bufs=3))
            qpool = ctx.enter_context(tc.tile_pool(name="qpool", bufs=4))
            xtpool = ctx.enter_context(tc.tile_pool(name="xtpool", bufs=6))
            ypool = ctx.enter_context(tc.tile_pool(name="ypool", bufs=3))
            stats = ctx.enter_context(tc.tile_pool(name="stats", bufs=3))
            pspool = ctx.enter_context(
                tc.tile_pool(name="pspool", bufs=2, space="PSUM")
            )
            wps = ctx.enter_context(
                tc.tile_pool(name="wps", bufs=2, space="PSUM")
            )

            # ---------------- constants ----------------
            identity = consts.tile([P, P], F32)
            make_identity(nc, identity)
            ones_row = consts.tile([1, P], BF16)
            nc.vector.memset(ones_row, 1.0)

            # ---------------- weight prep ----------------
            w_sb = wpool.tile([P, OCH, D], F32)
            nc.sync.dma_start(out=w_sb, in_=w_r)

            # stats: sum(w) on ACT (Copy with add-accumulate), sum|w| on DVE
            wsum = consts.tile([P, OCH], F32)
            wabs = consts.tile([P, OCH], F32)
            for r in range(OCH):
                dump = scr.tile([P, D], F32, tag="wdump")
                nc.scalar.activation(
                    out=dump, in_=w_sb[:, r, :], func=ACTF.Copy,
                    accum_out=wsum[:, r : r + 1],
                )
                nc.vector.tensor_reduce(
                    out=wabs[:, r : r + 1], in_=w_sb[:, r, :],
                    axis=mybir.AxisListType.X, op=ALU.add,
                    apply_absolute_value=True,
                )
            wsum1 = consts.tile([P, 1], F32)
            wabs1 = consts.tile([P, 1], F32)
            nc.vector.tensor_reduce(
                out=wsum1, in_=wsum, axis=mybir.AxisListType.X, op=ALU.add
            )
            nc.vector.tensor_reduce(
                out=wabs1, in_=wabs, axis=mybir.AxisListType.X, op=ALU.add
            )
            wsum_t = consts.tile([P, 1], F32)
            wabs_t = consts.tile([P, 1], F32)
            nc.gpsimd.partition_all_reduce(
                wsum_t, wsum1, channels=P, reduce_op=bass_isa.ReduceOp.add
            )
            nc.gpsimd.partition_all_reduce(
                wabs_t, wabs1, channels=P, reduce_op=bass_isa.ReduceOp.add
            )
            neg_mean_w = consts.tile([P, 1], F32)
            w_scale = consts.tile([P, 1], F32)
            nc.vector.tensor_scalar(
                out=neg_mean_w, in0=wsum_t, scalar1=-1.0 / float(O * D),
                scalar2=None, op0=ALU.mult,
            )
            nc.vector.tensor_scalar(
                out=w_scale, in0=wabs_t, scalar1=1.0 / float(O * D),
                scalar2=None, op0=ALU.mult,
            )

            # transpose raw w on PE (f32, no stats dependency), then
            # wqT[:, c, :] = Sign(wT_c - mean) on ACT straight from PSUM
            wqT = wtpool.tile([P, DCH, O], BF16)
            for c in range(DCH):
                pt = wps.tile([P, O], F32, tag="wtp")
                for r in range(OCH):
                    nc.tensor.transpose(
                        pt[:, r * P : (r + 1) * P],
                        w_sb[:, r, c * P : (c + 1) * P],
                        identity,
                    )
                nc.scalar.activation(
                    out=wqT[:, c, :], in_=pt, func=ACTF.Sign,
                    bias=neg_mean_w, scale=1.0,
                )

            # ---------------- bias prep ----------------
            b_sb = consts.tile([1, O], F32)
            nc.sync.dma_start(out=b_sb, in_=b_r)
            bsum = consts.tile([1, 1], F32)
            babs = consts.tile([1, 1], F32)
            nc.vector.tensor_reduce(
                out=bsum, in_=b_sb, axis=mybir.AxisListType.X, op=ALU.add
            )
            nc.vector.tensor_reduce(
                out=babs, in_=b_sb, axis=mybir.AxisListType.X, op=ALU.add,
                apply_absolute_value=True,
            )
            neg_mean_b = consts.tile([1, 1], F32)
            b_scale1 = consts.tile([1, 1], F32)
            nc.vector.tensor_scalar(
                out=neg_mean_b, in0=bsum, scalar1=-1.0 / float(O),
                scalar2=None, op0=ALU.mult,
            )
            nc.vector.tensor_scalar(
                out=b_scale1, in0=babs, scalar1=1.0 / float(O),
                scalar2=None, op0=ALU.mult,
            )
            bq = consts.tile([1, O], BF16)
            nc.scalar.activation(
                out=bq, in_=b_sb, func=ACTF.Sign, bias=neg_mean_b, scale=1.0
            )

            # invc = 1 / (127 * w_scale * b_scale), broadcast to [128,1]
            b_scale = consts.tile([P, 1], F32)
            nc.gpsimd.partition_broadcast(b_scale, b_scale1)
            wb = consts.tile([P, 1], F32)
            nc.vector.tensor_tensor(
                out=wb, in0=w_scale, in1=b_scale, op=ALU.mult
            )
            wb127 = consts.tile([P, 1], F32)
            nc.vector.tensor_scalar(
                out=wb127, in0=wb, scalar1=127.0, scalar2=None, op0=ALU.mult
            )
            invc = consts.tile([P, 1], F32)
            nc.vector.reciprocal(out=invc, in_=wb127)

            # ---------------- main loop ----------------
            # (optionally wrapped in a HW loop for benchmarking: each
            # iteration recomputes the same outputs, so repeat>1 is
            # idempotent and lets wall-clock differencing isolate the
            # steady-state loop time)
            def main_loop():
                for g in range(NGROUPS):
                    emit_group(g)

            def emit_group(g):
                xg = xpool.tile([P, GROUP, D], F32)
                nc.sync.dma_start(
                    out=xg, in_=x_r[:, g * GROUP : (g + 1) * GROUP, :]
                )

                sumsq = stats.tile([P, GROUP], F32)
                amax = stats.tile([P, GROUP], F32)
                for j in range(GROUP):
                    # sum(x^2) on ACT (Square with add-accumulate)
                    sq = scr.tile([P, D], F32, tag="sq")
                    nc.scalar.activation(
                        out=sq, in_=xg[:, j, :], func=ACTF.Square,
                        accum_out=sumsq[:, j : j + 1],
                    )
                    nc.vector.tensor_reduce(
                        out=amax[:, j : j + 1], in_=xg[:, j, :],
                        axis=mybir.AxisListType.X, op=ALU.max,
                        apply_absolute_value=True,
                    )

                # per-token scalar chain on [128, GROUP]
                ssq = stats.tile([P, GROUP], F32)
                nc.vector.tensor_scalar(
                    out=ssq, in0=sumsq, scalar1=EPS_NORM_SQ, scalar2=None,
                    op0=ALU.max,
                )
                u = stats.tile([P, GROUP], F32)
                nc.vector.reciprocal(out=u, in_=ssq)
                v = stats.tile([P, GROUP], F32)
                nc.scalar.activation(out=v, in_=u, func=ACTF.Sqrt)
                for _ in range(2):  # Newton rsqrt refinement
                    rr = stats.tile([P, GROUP], F32, tag="rr")
                    nc.vector.tensor_tensor(out=rr, in0=v, in1=v, op=ALU.mult)
                    qq = stats.tile([P, GROUP], F32, tag="qq")
                    nc.vector.tensor_tensor(out=qq, in0=rr, in1=ssq, op=ALU.mult)
                    ww = stats.tile([P, GROUP], F32, tag="ww")
                    nc.vector.tensor_scalar(
                        out=ww, in0=qq, scalar1=-0.5, scalar2=1.5,
                        op0=ALU.mult, op1=ALU.add,
                    )
                    v2 = stats.tile([P, GROUP], F32, tag="vv")
                    nc.vector.tensor_tensor(out=v2, in0=v, in1=ww, op=ALU.mult)
                    v = v2

                am = stats.tile([P, GROUP], F32)
                nc.vector.tensor_scalar(
                    out=am, in0=amax, scalar1=1e-30, scalar2=None, op0=ALU.max
                )
                im = stats.tile([P, GROUP], F32)
                nc.vector.reciprocal(out=im, in_=am)
                m = stats.tile([P, GROUP], F32)
                nc.vector.tensor_scalar(
                    out=m, in0=im, scalar1=127.0, scalar2=None, op0=ALU.mult
                )
                ax1 = stats.tile([P, GROUP], F32)
                nc.vector.tensor_tensor(out=ax1, in0=amax, in1=v, op=ALU.mult)
                axnc = stats.tile([P, GROUP], F32)
                nc.vector.tensor_scalar(
                    out=axnc, in0=ax1, scalar1=DIM_SCALE, scalar2=EPS_SCALE,
                    op0=ALU.mult, op1=ALU.max,
                )
                gsc = stats.tile([P, GROUP], F32)
                nc.vector.tensor_scalar(
                    out=gsc, in0=axnc, scalar1=invc, scalar2=None, op0=ALU.mult
                )

                for j in range(GROUP):
                    # quantize: x_q = round(x * m) via magic constant
                    t1 = tpool.tile([P, D], F32)
                    nc.vector.tensor_scalar(
                        out=t1, in0=xg[:, j, :], scalar1=m[:, j : j + 1],
                        scalar2=MAGIC, op0=ALU.mult, op1=ALU.add,
                    )
                    xq = qpool.tile([P, D], BF16)
                    nc.vector.tensor_scalar(
                        out=xq, in0=t1, scalar1=MAGIC, scalar2=None,
                        op0=ALU.subtract,
                    )

                    # transpose x_q -> [d-chunk][128, t] in one xbar DMA:
                    # xqT[p, c, t] = xq[t, c*128+p]
                    xqT = xtpool.tile([P, DCH, P], BF16)
                    if TRANSPOSE_MODE == "xbar":
                        nc.sync.dma_start_transpose(xqT, xq)
                    else:
                        ptx = wps.tile([P, D], BF16, tag="xtp")
                        for c in range(DCH):
                            nc.tensor.transpose(
                                ptx[:, c * P : (c + 1) * P],
                                xq[:, c * P : (c + 1) * P],
                                identity,
                            )
                        nc.vector.tensor_copy(
                            out=xqT.rearrange("p c t -> p (c t)"), in_=ptx
                        )

                    # matmul: y = x_q @ w_q.T + b_q  (PSUM f32, exact)
                    ps = pspool.tile([P, O], F32)
                    for h in range(2):
                        sl = slice(h * 512, (h + 1) * 512)
                        nc.tensor.matmul(
                            ps[:, sl], lhsT=ones_row, rhs=bq[:, sl],
                            start=True, stop=False,
                        )
                        for c in range(DCH):
                            nc.tensor.matmul(
                                ps[:, sl],
                                lhsT=xqT[:, c, :],
                                rhs=wqT[:, c, sl],
                                start=False, stop=(c == DCH - 1),
                            )

                    # dequant + store
                    yt = ypool.tile([P, O], F32)
                    nc.scalar.activation(
                        out=yt, in_=ps, func=ACTF.Copy, bias=0.0,
                        scale=gsc[:, j : j + 1],
                    )
                    nc.sync.dma_start(out=y_r[:, g * GROUP + j, :], in_=yt)

            if repeat == 1:
                main_loop()
            else:
                with tc.For_i(0, repeat, 1):
                    main_loop()

    nc.compile()
    return nc


_NC_CACHE = None


def _get_module():
    global _NC_CACHE
    if _NC_CACHE is None:
        _NC_CACHE = build_module()
    return _NC_CACHE


def kernel(x: np.ndarray, w: np.ndarray, b: np.ndarray) -> np.ndarray:
    assert x.shape == (B, S, D) and w.shape == (O, D) and b.shape == (O,)
    nc = _get_module()

    xf = np.ascontiguousarray(x.reshape(TOKENS, D), dtype=np.float32)
    w = np.ascontiguousarray(w, dtype=np.float32)
    b = np.ascontiguousarray(b, dtype=np.float32)

    in_maps = [
        {
            "x": xf[i * TOK_PER_CORE : (i + 1) * TOK_PER_CORE],
            "w": w,
            "b": b,
        }
        for i in range(N_CORES)
    ]
    res = run_bass_kernel_spmd(nc, in_maps, core_ids=list(range(N_CORES)))
    out = np.concatenate([res.results[i]["y"] for i in range(N_CORES)], axis=0)
    return out.reshape(B, S, O).astype(np.float32)
